# revision 6
# baseline (speedup 1.0000x reference)
"""Causal self-attention (B=4, S=4096, D=64, H=4) on 8 TRN2 NeuronCores.

Sharding: the 16 (batch, head) pairs are distributed 2-per-core
(core c -> batch c//2, heads (2*(c%2), 2*(c%2)+1)). Each core runs the
full fused attention for its 2 pairs; no cross-core communication.

Per-core bass program (SPMD, identical shapes on all cores):
  - inputs: xT_aug [65, 4096] bf16 (x[b].T plus a ones row so the QKV
    biases come in through the matmul), wqk [65, 64] bf16 (per-pair
    [Wq_aug | Wk_aug] columns, q pre-scaled by 1/sqrt(Dh)),
    wv [65, 32] bf16, mask [128, 128] f32 (strictly-lower-triangular
    -1e4 additive causal mask).
  - QKV projection on PE; scores computed TRANSPOSED (S.T = K_blk @ Q.T,
    key position on partitions) so that the P@V contraction needs no
    transpose of P; softmax denominator comes for free from a 17th
    all-ones column appended to V.  exp runs on the scalar engine
    straight out of PSUM; max-subtraction is skipped (scores are O(13),
    exp cannot overflow fp32).
  - output: [2 pairs, 17, 4096] f32 = unnormalized O.T rows 0..15 plus
    the softmax denominator in row 16; the division happens on host.
"""

import numpy as np
import ml_dtypes

_B, _S, _D = 4, 4096, 64
_H, _Dh = 4, 16
_NC = 8
_SCALE = 1.0 / np.sqrt(_Dh)
_MASK_NEG = -10000.0
_NQB = _S // 512  # 8 query super-blocks of 512
_NKB = _S // 128  # 32 key blocks of 128

_cache = {}


def _build_nc():
    import concourse.tile as tile
    from concourse import bacc, mybir

    bf = mybir.dt.bfloat16
    f32 = mybir.dt.float32
    Exp = mybir.ActivationFunctionType.Exp

    nc = bacc.Bacc("TRN2", target_bir_lowering=False, debug=False, num_devices=_NC)
    xT_d = nc.dram_tensor("xT", [_D + 1, _S], bf, kind="ExternalInput").ap()
    # 48 cols per pair: [q_aug | 16 zero cols | k_aug] so that in the
    # projection PSUM tile q sits at partitions 0..15 and k at 32..47
    # (PSUM reads must start at a 32-aligned partition).
    wqk_d = nc.dram_tensor("wqk", [_D + 1, 96], bf, kind="ExternalInput").ap()
    wv_d = nc.dram_tensor("wv", [_D + 1, 32], bf, kind="ExternalInput").ap()
    mask_d = nc.dram_tensor("mask", [128, 128], f32, kind="ExternalInput").ap()
    out_d = nc.dram_tensor("out", [2, 17, _S], f32, kind="ExternalOutput").ap()

    with tile.TileContext(nc) as tc:
        with tc.tile_pool(name="singles", bufs=1) as singles:
            xT = singles.tile([_D + 1, _S], bf, tag="xT")
            wqk = singles.tile([_D + 1, 96], bf, tag="wqk")
            wv = singles.tile([_D + 1, 32], bf, tag="wv")
            maskt = singles.tile([128, 128], f32, tag="mask")
            nc.sync.dma_start(out=xT[:], in_=xT_d)
            nc.sync.dma_start(out=wqk[:], in_=wqk_d)
            nc.sync.dma_start(out=wv[:], in_=wv_d)
            nc.sync.dma_start(out=maskt[:], in_=mask_d)

            qT = [singles.tile([16, _S], bf, tag=f"qT{p}", name=f"qT{p}") for p in range(2)]
            kT = [singles.tile([16, _S], bf, tag=f"kT{p}", name=f"kT{p}") for p in range(2)]
            V = [singles.tile([128, 17 * _NKB], bf, tag=f"V{p}", name=f"V{p}") for p in range(2)]
            outst = [singles.tile([17, _S], f32, tag=f"o{p}", name=f"ost{p}") for p in range(2)]
            for p in range(2):
                nc.vector.memset(V[p][:], 1.0)

            # ---- QKV projections ----
            with tc.tile_pool(name="ps_proj", bufs=2, space="PSUM") as psA:
                for p in range(2):
                    for c in range(_S // 512):
                        pq = psA.tile([48, 512], f32, tag="qk")
                        nc.tensor.matmul(
                            pq[:],
                            wqk[:, 48 * p : 48 * p + 48],
                            xT[:, 512 * c : 512 * (c + 1)],
                            start=True,
                            stop=True,
                        )
                        nc.vector.tensor_copy(
                            qT[p][:, 512 * c : 512 * (c + 1)], pq[0:16, :]
                        )
                        nc.vector.tensor_copy(
                            kT[p][:, 512 * c : 512 * (c + 1)], pq[32:48, :]
                        )
                for s in range(_NKB):
                    pv = psA.tile([128, 32], f32, tag="v")
                    nc.tensor.matmul(
                        pv[:],
                        xT[:, 128 * s : 128 * (s + 1)],
                        wv[:],
                        start=True,
                        stop=True,
                    )
                    for p in range(2):
                        nc.vector.tensor_copy(
                            V[p][:, 17 * s : 17 * s + 16],
                            pv[:, 16 * p : 16 * p + 16],
                        )

            # ---- attention ----
            with (
                tc.tile_pool(name="ps_sc", bufs=2, space="PSUM") as ps_sc,
                tc.tile_pool(name="ps_o", bufs=2, space="PSUM") as ps_o,
                tc.tile_pool(name="ptp", bufs=3) as ptp,
            ):
                for p in range(2):
                    for qi in range(_NQB):
                        nkb = 4 * qi + 4
                        po = ps_o.tile([17, 512], f32, tag="po")
                        for ck in range(nkb // 2):
                            ps = ps_sc.tile([128, 1024], f32, tag="sc")
                            pt = ptp.tile([128, 1024], bf, tag="pt")
                            for t in range(2):
                                b = 2 * ck + t
                                nc.tensor.matmul(
                                    ps[:, 512 * t : 512 * (t + 1)],
                                    kT[p][:, 128 * b : 128 * (b + 1)],
                                    qT[p][:, 512 * qi : 512 * (qi + 1)],
                                    start=True,
                                    stop=True,
                                )
                                j = b - 4 * qi
                                if j >= 0:  # diagonal block: causal mask
                                    sl = ps[:, 512 * t + 128 * j : 512 * t + 128 * (j + 1)]
                                    nc.vector.tensor_add(sl, sl, maskt[:])
                            nc.scalar.activation(out=pt[:], in_=ps[:], func=Exp)
                            for t in range(2):
                                b = 2 * ck + t
                                j = b - 4 * qi
                                qoff = 128 * j if j > 0 else 0
                                nc.tensor.matmul(
                                    po[:, qoff:512],
                                    V[p][:, 17 * b : 17 * b + 17],
                                    pt[:, 512 * t + qoff : 512 * (t + 1)],
                                    start=(b == 0),
                                    stop=(b == nkb - 1),
                                )
                        nc.vector.tensor_copy(
                            outst[p][:, 512 * qi : 512 * (qi + 1)], po[:]
                        )
                    nc.sync.dma_start(out=out_d[p], in_=outst[p][:])

    nc.compile()
    return nc


def _get_nc():
    if "nc" not in _cache:
        _cache["nc"] = _build_nc()
    return _cache["nc"]


def _prepare_in_maps(x, Wq, bq, Wk, bk, Wv, bv):
    bf = ml_dtypes.bfloat16
    x = np.asarray(x, np.float32)
    ones = np.ones((1, _S), np.float32)

    def aug(W, b, h, scale=1.0):
        # [Dh, D+1] block for head h: weight rows plus bias column
        blk = np.concatenate(
            [W[h * _Dh : (h + 1) * _Dh, :], b[h * _Dh : (h + 1) * _Dh, None]], axis=1
        )
        return (blk * scale).T.astype(np.float32)  # [D+1, Dh]

    mask = np.where(
        np.arange(128)[:, None] > np.arange(128)[None, :], _MASK_NEG, 0.0
    ).astype(np.float32)

    in_maps = []
    for c in range(_NC):
        b_idx = c // 2
        heads = (2 * (c % 2), 2 * (c % 2) + 1)
        xT = np.concatenate([x[b_idx].T, ones], axis=0)  # [65, 4096]
        wqk_cols = []
        wv_cols = []
        zeros16 = np.zeros((_D + 1, _Dh), np.float32)
        for h in heads:
            wqk_cols.append(aug(Wq, bq, h, _SCALE))
            wqk_cols.append(zeros16)
            wqk_cols.append(aug(Wk, bk, h))
            wv_cols.append(aug(Wv, bv, h))
        in_maps.append(
            {
                "xT": xT.astype(bf),
                "wqk": np.concatenate(wqk_cols, axis=1).astype(bf),
                "wv": np.concatenate(wv_cols, axis=1).astype(bf),
                "mask": mask,
            }
        )
    return in_maps


def _assemble(results):
    final = np.empty((_B, _S, _D), np.float32)
    for c in range(_NC):
        b_idx = c // 2
        for p in range(2):
            h = 2 * (c % 2) + p
            o = np.asarray(results[c]["out"], np.float32)  # [2, 17, S]
            final[b_idx, :, h * _Dh : (h + 1) * _Dh] = (o[p, :16] / o[p, 16:17]).T
    return final


def _run(in_maps, trace=False, trace_kwargs=None):
    from concourse.bass_utils import run_bass_kernel_spmd

    nc = _get_nc()
    return run_bass_kernel_spmd(
        nc, in_maps, list(range(_NC)), trace=trace, **(trace_kwargs or {})
    )


def kernel(x, Wq, bq, Wk, bk, Wv, bv):
    in_maps = _prepare_in_maps(x, Wq, bq, Wk, bk, Wv, bv)
    res = _run(in_maps)
    return _assemble(res.results)


# revision 7
# speedup vs baseline: 1.4595x; 1.4595x over previous
"""Causal self-attention (B=4, S=4096, D=64, H=4) on 8 TRN2 NeuronCores.

Sharding: the 16 (batch, head) pairs are distributed 2-per-core
(core c -> batch c//2, heads (2*(c%2), 2*(c%2)+1)). Each core runs the
full fused attention for its 2 pairs; no cross-core communication.

Per-core bass program (SPMD, identical shapes on all cores):
  - inputs: xT_aug [65, 4096] bf16 (x[b].T plus a ones row so the QKV
    biases come in through the matmul), wqk [65, 96] bf16 (per-pair
    [Wq_aug | 16 zero cols | Wk_aug] columns, q pre-scaled by
    1/sqrt(Dh); the zero gap puts k at a 32-aligned PSUM partition),
    wv [65, 32] bf16, mask [128, 128] f32 (strictly-lower-triangular
    -1e4 additive causal mask).
  - scores are computed TRANSPOSED (S.T = K_blk @ Q.T, key position on
    partitions) so the P@V contraction needs no transpose of P; the
    softmax denominator comes free from a 17th all-ones column in V.
    exp runs on the scalar engine straight out of PSUM; max-subtraction
    is skipped (scores are O(13), exp cannot overflow fp32).
  - The PE's HAM clock gate treats K=16 matmuls as idle and throttles
    to 1.2 GHz, so per query super-block the kernel runs phase A (all
    score matmuls, 4-way row-tiled via tile_position with qT/kT
    replicated at partition offsets 0/32/64/96 -> ~4x concurrency,
    bursts too short to re-throttle) then phase B (K=128 PV matmuls,
    which count as busy and keep the clock warm).
  - output: [2 pairs, 17, 4096] f32 = unnormalized O.T rows 0..15 plus
    the softmax denominator in row 16; the division happens on host.
"""

import numpy as np
import ml_dtypes

_B, _S, _D = 4, 4096, 64
_H, _Dh = 4, 16
_NC = 8
_SCALE = 1.0 / np.sqrt(_Dh)
_MASK_NEG = -10000.0
_NQB = _S // 512  # 8 query super-blocks of 512
_NKB = _S // 128  # 32 key blocks of 128
_CHUNK = 3  # k-blocks per exp chunk (3 PSUM banks)

_cache = {}


def _build_nc():
    import concourse.tile as tile
    from concourse import bacc, mybir

    bf = mybir.dt.bfloat16
    f32 = mybir.dt.float32
    Exp = mybir.ActivationFunctionType.Exp

    nc = bacc.Bacc("TRN2", target_bir_lowering=False, debug=False, num_devices=_NC)
    xT_d = nc.dram_tensor("xT", [_D + 1, _S], bf, kind="ExternalInput").ap()
    wqk_d = nc.dram_tensor("wqk", [_D + 1, 96], bf, kind="ExternalInput").ap()
    wv_d = nc.dram_tensor("wv", [_D + 1, 32], bf, kind="ExternalInput").ap()
    mask_d = nc.dram_tensor("mask", [128, 128], f32, kind="ExternalInput").ap()
    out_d = nc.dram_tensor("out", [2, 17, _S], f32, kind="ExternalOutput").ap()

    with tile.TileContext(nc) as tc:
        with tc.tile_pool(name="singles", bufs=1) as singles:
            xT = singles.tile([_D + 1, _S], bf, tag="xT")
            wqk = singles.tile([_D + 1, 96], bf, tag="wqk")
            wv = singles.tile([_D + 1, 32], bf, tag="wv")
            maskt = singles.tile([128, 128], f32, tag="mask")
            nc.sync.dma_start(out=xT[:], in_=xT_d)
            nc.sync.dma_start(out=wqk[:], in_=wqk_d)
            nc.sync.dma_start(out=wv[:], in_=wv_d)
            nc.sync.dma_start(out=maskt[:], in_=mask_d)

            # qT/kT replicated at partition offsets 0/32/64/96 for 4-way
            # row-tiled score matmuls.
            qT = [singles.tile([128, _S], bf, tag=f"qT{p}", name=f"qT{p}") for p in range(2)]
            kT = [singles.tile([128, _S], bf, tag=f"kT{p}", name=f"kT{p}") for p in range(2)]
            V = [singles.tile([128, 17 * _NKB], bf, tag=f"V{p}", name=f"V{p}") for p in range(2)]
            for p in range(2):
                nc.vector.memset(V[p][:], 1.0)

            # ---- QKV projections ----
            with tc.tile_pool(name="ps_proj", bufs=2, space="PSUM") as psA:
                for p in range(2):
                    for c in range(_S // 512):
                        pq = psA.tile([48, 512], f32, tag="qk")
                        nc.tensor.matmul(
                            pq[:],
                            wqk[:, 48 * p : 48 * p + 48],
                            xT[:, 512 * c : 512 * (c + 1)],
                            start=True,
                            stop=True,
                        )
                        nc.vector.tensor_copy(
                            qT[p][0:16, 512 * c : 512 * (c + 1)], pq[0:16, :]
                        )
                        nc.vector.tensor_copy(
                            kT[p][0:16, 512 * c : 512 * (c + 1)], pq[32:48, :]
                        )
                for s in range(_NKB):
                    pv = psA.tile([128, 32], f32, tag="v")
                    nc.tensor.matmul(
                        pv[:],
                        xT[:, 128 * s : 128 * (s + 1)],
                        wv[:],
                        start=True,
                        stop=True,
                    )
                    for p in range(2):
                        nc.vector.tensor_copy(
                            V[p][:, 17 * s : 17 * s + 16],
                            pv[:, 16 * p : 16 * p + 16],
                        )
                # replicate qT/kT rows 0..15 to partition offsets 32/64/96
                for p in range(2):
                    for g in range(1, 4):
                        nc.sync.dma_start(
                            out=qT[p][32 * g : 32 * g + 16, :], in_=qT[p][0:16, :]
                        )
                        nc.sync.dma_start(
                            out=kT[p][32 * g : 32 * g + 16, :], in_=kT[p][0:16, :]
                        )

            # ---- attention ----
            with (
                tc.tile_pool(name="ps_sc", bufs=2, space="PSUM") as ps_sc,
                tc.tile_pool(name="ps_o", bufs=2, space="PSUM") as ps_o,
                tc.tile_pool(name="ptp", bufs=2) as ptp,
                tc.tile_pool(name="stg", bufs=3) as stg,
            ):
                for p in range(2):
                    for qi in range(_NQB):
                        nkb = 4 * qi + 4
                        qsl = slice(512 * qi, 512 * (qi + 1))
                        pt = ptp.tile([128, 512 * _NKB], bf, tag="pt")
                        # --- phase A: scores (row-tiled) + exp ---
                        b0 = 0
                        while b0 < nkb:
                            nblk = min(_CHUNK, nkb - b0)
                            ps = ps_sc.tile([128, 512 * _CHUNK], f32, tag="sc")
                            for t in range(nblk):
                                b = b0 + t
                                g = b % 4
                                nc.tensor.matmul(
                                    ps[:, 512 * t : 512 * (t + 1)],
                                    kT[p][32 * g : 32 * g + 16, 128 * b : 128 * (b + 1)],
                                    qT[p][32 * g : 32 * g + 16, qsl],
                                    start=True,
                                    stop=True,
                                    tile_position=(32 * g, 0),
                                )
                                j = b - 4 * qi
                                if j >= 0:  # diagonal block: causal mask
                                    sl = ps[
                                        :, 512 * t + 128 * j : 512 * t + 128 * (j + 1)
                                    ]
                                    nc.vector.tensor_add(sl, sl, maskt[:])
                            nc.scalar.activation(
                                out=pt[:, 512 * b0 : 512 * (b0 + nblk)],
                                in_=ps[:, : 512 * nblk],
                                func=Exp,
                            )
                            b0 += nblk
                        # --- phase B: PV (K=128 keeps the PE clock warm) ---
                        po = ps_o.tile([17, 512], f32, tag="po")
                        for b in range(nkb):
                            j = b - 4 * qi
                            qoff = 128 * j if j > 0 else 0
                            nc.tensor.matmul(
                                po[:, qoff:512],
                                V[p][:, 17 * b : 17 * b + 17],
                                pt[:, 512 * b + qoff : 512 * (b + 1)],
                                start=(b == 0),
                                stop=(b == nkb - 1),
                            )
                        ost = stg.tile([17, 512], f32, tag="ost")
                        nc.vector.tensor_copy(ost[:], po[:])
                        nc.sync.dma_start(out=out_d[p][:, qsl], in_=ost[:])

    nc.compile()
    return nc


def _get_nc():
    if "nc" not in _cache:
        _cache["nc"] = _build_nc()
    return _cache["nc"]


def _prepare_in_maps(x, Wq, bq, Wk, bk, Wv, bv):
    bf = ml_dtypes.bfloat16
    x = np.asarray(x, np.float32)
    ones = np.ones((1, _S), np.float32)

    def aug(W, b, h, scale=1.0):
        # [Dh, D+1] block for head h: weight rows plus bias column
        blk = np.concatenate(
            [W[h * _Dh : (h + 1) * _Dh, :], b[h * _Dh : (h + 1) * _Dh, None]], axis=1
        )
        return (blk * scale).T.astype(np.float32)  # [D+1, Dh]

    mask = np.where(
        np.arange(128)[:, None] > np.arange(128)[None, :], _MASK_NEG, 0.0
    ).astype(np.float32)

    in_maps = []
    for c in range(_NC):
        b_idx = c // 2
        heads = (2 * (c % 2), 2 * (c % 2) + 1)
        xT = np.concatenate([x[b_idx].T, ones], axis=0)  # [65, 4096]
        wqk_cols = []
        wv_cols = []
        zeros16 = np.zeros((_D + 1, _Dh), np.float32)
        for h in heads:
            wqk_cols.append(aug(Wq, bq, h, _SCALE))
            wqk_cols.append(zeros16)
            wqk_cols.append(aug(Wk, bk, h))
            wv_cols.append(aug(Wv, bv, h))
        in_maps.append(
            {
                "xT": xT.astype(bf),
                "wqk": np.concatenate(wqk_cols, axis=1).astype(bf),
                "wv": np.concatenate(wv_cols, axis=1).astype(bf),
                "mask": mask,
            }
        )
    return in_maps


def _assemble(results):
    final = np.empty((_B, _S, _D), np.float32)
    for c in range(_NC):
        b_idx = c // 2
        for p in range(2):
            h = 2 * (c % 2) + p
            o = np.asarray(results[c]["out"], np.float32)  # [2, 17, S]
            final[b_idx, :, h * _Dh : (h + 1) * _Dh] = (o[p, :16] / o[p, 16:17]).T
    return final


def _run(in_maps, trace=False, trace_kwargs=None):
    from concourse.bass_utils import run_bass_kernel_spmd

    nc = _get_nc()
    return run_bass_kernel_spmd(
        nc, in_maps, list(range(_NC)), trace=trace, **(trace_kwargs or {})
    )


def kernel(x, Wq, bq, Wk, bk, Wv, bv):
    in_maps = _prepare_in_maps(x, Wq, bq, Wk, bk, Wv, bv)
    res = _run(in_maps)
    return _assemble(res.results)


# revision 8
# speedup vs baseline: 1.4913x; 1.0218x over previous
"""Causal self-attention (B=4, S=4096, D=64, H=4) on 8 TRN2 NeuronCores.

Sharding: the 16 (batch, head) pairs are distributed 2-per-core
(core c -> batch c//2, heads (2*(c%2), 2*(c%2)+1)). Each core runs the
full fused attention for its 2 pairs; no cross-core communication.

Per-core bass program (SPMD, identical shapes on all cores):
  - inputs: xT_aug [65, 4096] bf16 (x[b].T plus a ones row so the QKV
    biases come in through the matmul), wqk [65, 96] bf16 (per-pair
    [Wq_aug | 16 zero cols | Wk_aug] columns, q pre-scaled by
    1/sqrt(Dh); the zero gap puts k at a 32-aligned PSUM partition),
    wv [65, 32] bf16, mask [128, 128] f32 (strictly-lower-triangular
    -1e4 additive causal mask).
  - scores are computed TRANSPOSED (S.T = K_blk @ Q.T, key position on
    partitions) so the P@V contraction needs no transpose of P; the
    softmax denominator comes free from a 17th all-ones column in V.
    exp runs on the scalar engine straight out of PSUM; max-subtraction
    is skipped (scores are O(13), exp cannot overflow fp32).
  - The PE's HAM clock gate treats K=16 matmuls as idle and throttles
    to 1.2 GHz, so per query super-block the kernel runs phase A (all
    score matmuls, 4-way row-tiled via tile_position with qT/kT
    replicated at partition offsets 0/32/64/96 -> ~4x concurrency,
    bursts too short to re-throttle) then phase B (K=128 PV matmuls,
    which count as busy and keep the clock warm).
  - output: [2 pairs, 17, 4096] f32 = unnormalized O.T rows 0..15 plus
    the softmax denominator in row 16; the division happens on host.
"""

import numpy as np
import ml_dtypes

_B, _S, _D = 4, 4096, 64
_H, _Dh = 4, 16
_NC = 8
_SCALE = 1.0 / np.sqrt(_Dh)
_MASK_NEG = -10000.0
_NQB = _S // 512  # 8 query super-blocks of 512
_NKB = _S // 128  # 32 key blocks of 128
_CHUNK = 3  # k-blocks per exp chunk (3 PSUM banks)

_cache = {}


def _build_nc():
    import concourse.tile as tile
    from concourse import bacc, mybir

    bf = mybir.dt.bfloat16
    f32 = mybir.dt.float32
    Exp = mybir.ActivationFunctionType.Exp

    nc = bacc.Bacc("TRN2", target_bir_lowering=False, debug=False, num_devices=_NC)
    xT_d = nc.dram_tensor("xT", [_D + 1, _S], bf, kind="ExternalInput").ap()
    wqk_d = nc.dram_tensor("wqk", [_D + 1, 96], bf, kind="ExternalInput").ap()
    wv_d = nc.dram_tensor("wv", [_D + 1, 32], bf, kind="ExternalInput").ap()
    mask_d = nc.dram_tensor("mask", [128, 128], f32, kind="ExternalInput").ap()
    out_d = nc.dram_tensor("out", [2, 17, _S], f32, kind="ExternalOutput").ap()

    with tile.TileContext(nc) as tc:
        with tc.tile_pool(name="singles", bufs=1) as singles:
            xT = singles.tile([_D + 1, _S], bf, tag="xT")
            wqk = singles.tile([_D + 1, 96], bf, tag="wqk")
            wv = singles.tile([_D + 1, 32], bf, tag="wv")
            maskt = singles.tile([128, 128], f32, tag="mask")
            nc.sync.dma_start(out=xT[:], in_=xT_d)
            nc.sync.dma_start(out=wqk[:], in_=wqk_d)
            nc.sync.dma_start(out=wv[:], in_=wv_d)
            nc.sync.dma_start(out=maskt[:], in_=mask_d)

            # qT/kT replicated at partition offsets 0/32/64/96 for 4-way
            # row-tiled score matmuls.
            qT = [singles.tile([128, _S], bf, tag=f"qT{p}", name=f"qT{p}") for p in range(2)]
            kT = [singles.tile([128, _S], bf, tag=f"kT{p}", name=f"kT{p}") for p in range(2)]
            V = [singles.tile([128, 17 * _NKB], bf, tag=f"V{p}", name=f"V{p}") for p in range(2)]
            for p in range(2):
                nc.vector.memset(V[p][:], 1.0)

            # ---- QKV projections ----
            with tc.tile_pool(name="ps_proj", bufs=2, space="PSUM") as psA:
                for p in range(2):
                    for c in range(_S // 512):
                        pq = psA.tile([48, 512], f32, tag="qk")
                        nc.tensor.matmul(
                            pq[:],
                            wqk[:, 48 * p : 48 * p + 48],
                            xT[:, 512 * c : 512 * (c + 1)],
                            start=True,
                            stop=True,
                        )
                        nc.vector.tensor_copy(
                            qT[p][0:16, 512 * c : 512 * (c + 1)], pq[0:16, :]
                        )
                        nc.vector.tensor_copy(
                            kT[p][0:16, 512 * c : 512 * (c + 1)], pq[32:48, :]
                        )
                for s in range(_NKB):
                    pv = psA.tile([128, 32], f32, tag="v")
                    nc.tensor.matmul(
                        pv[:],
                        xT[:, 128 * s : 128 * (s + 1)],
                        wv[:],
                        start=True,
                        stop=True,
                    )
                    for p in range(2):
                        nc.vector.tensor_copy(
                            V[p][:, 17 * s : 17 * s + 16],
                            pv[:, 16 * p : 16 * p + 16],
                        )
                # replicate qT/kT rows 0..15 to partition offsets 32/64/96
                for p in range(2):
                    for g in range(1, 4):
                        nc.sync.dma_start(
                            out=qT[p][32 * g : 32 * g + 16, :], in_=qT[p][0:16, :]
                        )
                        nc.sync.dma_start(
                            out=kT[p][32 * g : 32 * g + 16, :], in_=kT[p][0:16, :]
                        )

            # ---- attention ----
            with (
                tc.tile_pool(name="ps_sc", bufs=2, space="PSUM") as ps_sc,
                tc.tile_pool(name="ps_o", bufs=2, space="PSUM") as ps_o,
                tc.tile_pool(name="ptp", bufs=2) as ptp,
                tc.tile_pool(name="stg", bufs=3) as stg,
            ):
                def phase_a(p, qi):
                    """Row-tiled score matmuls + causal mask + exp.
                    Returns the bf16 P.T tile for phase B."""
                    nkb = 4 * qi + 4
                    qsl = slice(512 * qi, 512 * (qi + 1))
                    pt = ptp.tile([128, 512 * _NKB], bf, tag="pt", name="pt")
                    b0 = 0
                    while b0 < nkb:
                        nblk = min(_CHUNK, nkb - b0)
                        ps = ps_sc.tile([128, 512 * _CHUNK], f32, tag="sc", name="ps")
                        for t in range(nblk):
                            b = b0 + t
                            g = b % 4
                            nc.tensor.matmul(
                                ps[:, 512 * t : 512 * (t + 1)],
                                kT[p][32 * g : 32 * g + 16, 128 * b : 128 * (b + 1)],
                                qT[p][32 * g : 32 * g + 16, qsl],
                                start=True,
                                stop=True,
                                tile_position=(32 * g, 0),
                            )
                            j = b - 4 * qi
                            if j >= 0:  # diagonal block: causal mask
                                sl = ps[
                                    :, 512 * t + 128 * j : 512 * t + 128 * (j + 1)
                                ]
                                nc.vector.tensor_add(sl, sl, maskt[:])
                        nc.scalar.activation(
                            out=pt[:, 512 * b0 : 512 * (b0 + nblk)],
                            in_=ps[:, : 512 * nblk],
                            func=Exp,
                        )
                        b0 += nblk
                    return pt

                def phase_b(p, qi, pt):
                    """PV matmuls (K=128 keeps the PE clock warm) + store."""
                    nkb = 4 * qi + 4
                    qsl = slice(512 * qi, 512 * (qi + 1))
                    po = ps_o.tile([17, 512], f32, tag="po", name="po")
                    for b in range(nkb):
                        j = b - 4 * qi
                        qoff = 128 * j if j > 0 else 0
                        nc.tensor.matmul(
                            po[:, qoff:512],
                            V[p][:, 17 * b : 17 * b + 17],
                            pt[:, 512 * b + qoff : 512 * (b + 1)],
                            start=(b == 0),
                            stop=(b == nkb - 1),
                        )
                    ost = stg.tile([17, 512], f32, tag="ost", name="ost")
                    nc.vector.tensor_copy(ost[:], po[:])
                    nc.sync.dma_start(out=out_d[p][:, qsl], in_=ost[:])

                # Software pipeline one unit ahead: A(i+1) is emitted before
                # B(i) so ACT exps unit i+1's scores while PE streams unit
                # i's PV matmuls.
                units = [(p, qi) for p in range(2) for qi in range(_NQB)]
                prev = None
                for p, qi in units:
                    pt = phase_a(p, qi)
                    if prev is not None:
                        phase_b(*prev)
                    prev = (p, qi, pt)
                phase_b(*prev)

    nc.compile()
    return nc


def _get_nc():
    if "nc" not in _cache:
        _cache["nc"] = _build_nc()
    return _cache["nc"]


def _prepare_in_maps(x, Wq, bq, Wk, bk, Wv, bv):
    bf = ml_dtypes.bfloat16
    x = np.asarray(x, np.float32)
    ones = np.ones((1, _S), np.float32)

    def aug(W, b, h, scale=1.0):
        # [Dh, D+1] block for head h: weight rows plus bias column
        blk = np.concatenate(
            [W[h * _Dh : (h + 1) * _Dh, :], b[h * _Dh : (h + 1) * _Dh, None]], axis=1
        )
        return (blk * scale).T.astype(np.float32)  # [D+1, Dh]

    mask = np.where(
        np.arange(128)[:, None] > np.arange(128)[None, :], _MASK_NEG, 0.0
    ).astype(np.float32)

    in_maps = []
    for c in range(_NC):
        b_idx = c // 2
        heads = (2 * (c % 2), 2 * (c % 2) + 1)
        xT = np.concatenate([x[b_idx].T, ones], axis=0)  # [65, 4096]
        wqk_cols = []
        wv_cols = []
        zeros16 = np.zeros((_D + 1, _Dh), np.float32)
        for h in heads:
            wqk_cols.append(aug(Wq, bq, h, _SCALE))
            wqk_cols.append(zeros16)
            wqk_cols.append(aug(Wk, bk, h))
            wv_cols.append(aug(Wv, bv, h))
        in_maps.append(
            {
                "xT": xT.astype(bf),
                "wqk": np.concatenate(wqk_cols, axis=1).astype(bf),
                "wv": np.concatenate(wv_cols, axis=1).astype(bf),
                "mask": mask,
            }
        )
    return in_maps


def _assemble(results):
    final = np.empty((_B, _S, _D), np.float32)
    for c in range(_NC):
        b_idx = c // 2
        for p in range(2):
            h = 2 * (c % 2) + p
            o = np.asarray(results[c]["out"], np.float32)  # [2, 17, S]
            final[b_idx, :, h * _Dh : (h + 1) * _Dh] = (o[p, :16] / o[p, 16:17]).T
    return final


def _run(in_maps, trace=False, trace_kwargs=None):
    from concourse.bass_utils import run_bass_kernel_spmd

    nc = _get_nc()
    return run_bass_kernel_spmd(
        nc, in_maps, list(range(_NC)), trace=trace, **(trace_kwargs or {})
    )


def kernel(x, Wq, bq, Wk, bk, Wv, bv):
    in_maps = _prepare_in_maps(x, Wq, bq, Wk, bk, Wv, bv)
    res = _run(in_maps)
    return _assemble(res.results)


# revision 9
# speedup vs baseline: 1.4981x; 1.0045x over previous
"""Causal self-attention (B=4, S=4096, D=64, H=4) on 8 TRN2 NeuronCores.

Sharding: the 16 (batch, head) pairs are distributed 2-per-core
(core c -> batch c//2, heads (2*(c%2), 2*(c%2)+1)). Each core runs the
full fused attention for its 2 pairs; no cross-core communication.

Per-core bass program (SPMD, identical shapes on all cores):
  - inputs: xT_aug [65, 4096] bf16 (x[b].T plus a ones row so the QKV
    biases come in through the matmul), wqk [65, 96] bf16 (per-pair
    [Wq_aug | 16 zero cols | Wk_aug] columns, q pre-scaled by
    1/sqrt(Dh); the zero gap puts k at a 32-aligned PSUM partition),
    wv [65, 32] bf16, mask [128, 128] f32 (strictly-lower-triangular
    -1e4 additive causal mask).
  - scores are computed TRANSPOSED (S.T = K_blk @ Q.T, key position on
    partitions) so the P@V contraction needs no transpose of P; the
    softmax denominator comes free from a 17th all-ones column in V.
    exp runs on the scalar engine straight out of PSUM; max-subtraction
    is skipped (scores are O(13), exp cannot overflow fp32).
  - The PE's HAM clock gate treats K=16 matmuls as idle and throttles
    to 1.2 GHz, so per query super-block the kernel runs phase A (all
    score matmuls, 4-way row-tiled via tile_position with qT/kT
    replicated at partition offsets 0/32/64/96 -> ~4x concurrency,
    bursts too short to re-throttle) then phase B (K=128 PV matmuls,
    which count as busy and keep the clock warm).
  - output: [2 pairs, 17, 4096] f32 = unnormalized O.T rows 0..15 plus
    the softmax denominator in row 16; the division happens on host.
"""

import numpy as np
import ml_dtypes

_B, _S, _D = 4, 4096, 64
_H, _Dh = 4, 16
_NC = 8
_SCALE = 1.0 / np.sqrt(_Dh)
_MASK_NEG = -10000.0
_NQB = _S // 512  # 8 query super-blocks of 512
_NKB = _S // 128  # 32 key blocks of 128
_CHUNK = 3  # k-blocks per exp chunk (3 PSUM banks)

_cache = {}


def _build_nc():
    import concourse.tile as tile
    from concourse import bacc, mybir

    bf = mybir.dt.bfloat16
    f32 = mybir.dt.float32
    Exp = mybir.ActivationFunctionType.Exp

    nc = bacc.Bacc("TRN2", target_bir_lowering=False, debug=False, num_devices=_NC)
    xT_d = nc.dram_tensor("xT", [_D + 1, _S], bf, kind="ExternalInput").ap()
    wqk_d = nc.dram_tensor("wqk", [_D + 1, 96], bf, kind="ExternalInput").ap()
    wv_d = nc.dram_tensor("wv", [_D + 1, 32], bf, kind="ExternalInput").ap()
    mask_d = nc.dram_tensor("mask", [128, 128], f32, kind="ExternalInput").ap()
    out_d = nc.dram_tensor("out", [2, 17, _S], f32, kind="ExternalOutput").ap()

    with tile.TileContext(nc) as tc:
        with tc.tile_pool(name="singles", bufs=1) as singles:
            xT = singles.tile([_D + 1, _S], bf, tag="xT")
            wqk = singles.tile([_D + 1, 96], bf, tag="wqk")
            wv = singles.tile([_D + 1, 32], bf, tag="wv")
            maskt = singles.tile([128, 128], f32, tag="mask")
            nc.sync.dma_start(out=xT[:], in_=xT_d)
            nc.sync.dma_start(out=wqk[:], in_=wqk_d)
            nc.sync.dma_start(out=wv[:], in_=wv_d)
            nc.sync.dma_start(out=maskt[:], in_=mask_d)

            # qT/kT replicated at partition offsets 0/32/64/96 for 4-way
            # row-tiled score matmuls.
            qT = [singles.tile([128, _S], bf, tag=f"qT{p}", name=f"qT{p}") for p in range(2)]
            kT = [singles.tile([128, _S], bf, tag=f"kT{p}", name=f"kT{p}") for p in range(2)]
            V = [singles.tile([128, 17 * _NKB], bf, tag=f"V{p}", name=f"V{p}") for p in range(2)]
            for p in range(2):
                nc.vector.memset(V[p][:], 1.0)

            # ---- QKV projections ----
            with tc.tile_pool(name="ps_proj", bufs=2, space="PSUM") as psA:
                for p in range(2):
                    for c in range(_S // 512):
                        pq = psA.tile([48, 512], f32, tag="qk")
                        nc.tensor.matmul(
                            pq[:],
                            wqk[:, 48 * p : 48 * p + 48],
                            xT[:, 512 * c : 512 * (c + 1)],
                            start=True,
                            stop=True,
                        )
                        nc.vector.tensor_copy(
                            qT[p][0:16, 512 * c : 512 * (c + 1)], pq[0:16, :]
                        )
                        nc.vector.tensor_copy(
                            kT[p][0:16, 512 * c : 512 * (c + 1)], pq[32:48, :]
                        )
                for s in range(_NKB):
                    pv = psA.tile([128, 32], f32, tag="v")
                    nc.tensor.matmul(
                        pv[:],
                        xT[:, 128 * s : 128 * (s + 1)],
                        wv[:],
                        start=True,
                        stop=True,
                    )
                    for p in range(2):
                        nc.vector.tensor_copy(
                            V[p][:, 17 * s : 17 * s + 16],
                            pv[:, 16 * p : 16 * p + 16],
                        )
                # replicate qT/kT rows 0..15 to partition offsets 32/64/96
                for p in range(2):
                    for g in range(1, 4):
                        nc.sync.dma_start(
                            out=qT[p][32 * g : 32 * g + 16, :], in_=qT[p][0:16, :]
                        )
                        nc.sync.dma_start(
                            out=kT[p][32 * g : 32 * g + 16, :], in_=kT[p][0:16, :]
                        )

            # ---- attention ----
            with (
                tc.tile_pool(name="ps_sc", bufs=2, space="PSUM") as ps_sc,
                tc.tile_pool(name="ps_o", bufs=2, space="PSUM") as ps_o,
                tc.tile_pool(name="ptp", bufs=2) as ptp,
                tc.tile_pool(name="stg", bufs=3) as stg,
            ):
                def emit_score_chunk(p, qi, pt, b0):
                    """One chunk of row-tiled score matmuls + mask + exp."""
                    nkb = 4 * qi + 4
                    qsl = slice(512 * qi, 512 * (qi + 1))
                    nblk = min(_CHUNK, nkb - b0)
                    ps = ps_sc.tile([128, 512 * _CHUNK], f32, tag="sc", name="ps")
                    for t in range(nblk):
                        b = b0 + t
                        g = b % 4
                        nc.tensor.matmul(
                            ps[:, 512 * t : 512 * (t + 1)],
                            kT[p][32 * g : 32 * g + 16, 128 * b : 128 * (b + 1)],
                            qT[p][32 * g : 32 * g + 16, qsl],
                            start=True,
                            stop=True,
                            tile_position=(32 * g, 0),
                        )
                        j = b - 4 * qi
                        if j >= 0:  # diagonal block: causal mask
                            sl = ps[:, 512 * t + 128 * j : 512 * t + 128 * (j + 1)]
                            nc.vector.tensor_add(sl, sl, maskt[:])
                    nc.scalar.activation(
                        out=pt[:, 512 * b0 : 512 * (b0 + nblk)],
                        in_=ps[:, : 512 * nblk],
                        func=Exp,
                    )

                class BUnit:
                    """PV matmuls (K=128 keeps the PE clock warm) + store,
                    emitted incrementally so they interleave with the next
                    unit's score chunks."""

                    def __init__(self, p, qi, pt):
                        self.p, self.qi, self.pt = p, qi, pt
                        self.nkb = 4 * qi + 4
                        self.done = 0
                        self.po = ps_o.tile([17, 512], f32, tag="po", name="po")

                    def emit_upto(self, k):
                        for b in range(self.done, min(k, self.nkb)):
                            j = b - 4 * self.qi
                            qoff = 128 * j if j > 0 else 0
                            nc.tensor.matmul(
                                self.po[:, qoff:512],
                                V[self.p][:, 17 * b : 17 * b + 17],
                                self.pt[:, 512 * b + qoff : 512 * (b + 1)],
                                start=(b == 0),
                                stop=(b == self.nkb - 1),
                            )
                        self.done = max(self.done, min(k, self.nkb))

                    def finish(self):
                        self.emit_upto(self.nkb)
                        qsl = slice(512 * self.qi, 512 * (self.qi + 1))
                        ost = stg.tile([17, 512], f32, tag="ost", name="ost")
                        nc.vector.tensor_copy(ost[:], self.po[:])
                        nc.sync.dma_start(out=out_d[self.p][:, qsl], in_=ost[:])

                # Fine-grained software pipeline: while emitting unit i's
                # score chunks (paced by ACT exp), interleave unit i-1's PV
                # matmuls proportionally so the PE never idles and the PV
                # density keeps the HAM clock gate warm.
                units = [(p, qi) for qi in range(_NQB) for p in range(2)]
                prev = None
                for p, qi in units:
                    nkb = 4 * qi + 4
                    nchunks = (nkb + _CHUNK - 1) // _CHUNK
                    pt = ptp.tile([128, 512 * _NKB], bf, tag="pt", name="pt")
                    for c in range(nchunks):
                        emit_score_chunk(p, qi, pt, c * _CHUNK)
                        if prev is not None:
                            prev.emit_upto(((c + 1) * prev.nkb) // nchunks)
                    if prev is not None:
                        prev.finish()
                    prev = BUnit(p, qi, pt)
                prev.finish()

    nc.compile()
    return nc


def _get_nc():
    if "nc" not in _cache:
        _cache["nc"] = _build_nc()
    return _cache["nc"]


def _prepare_in_maps(x, Wq, bq, Wk, bk, Wv, bv):
    bf = ml_dtypes.bfloat16
    x = np.asarray(x, np.float32)
    ones = np.ones((1, _S), np.float32)

    def aug(W, b, h, scale=1.0):
        # [Dh, D+1] block for head h: weight rows plus bias column
        blk = np.concatenate(
            [W[h * _Dh : (h + 1) * _Dh, :], b[h * _Dh : (h + 1) * _Dh, None]], axis=1
        )
        return (blk * scale).T.astype(np.float32)  # [D+1, Dh]

    mask = np.where(
        np.arange(128)[:, None] > np.arange(128)[None, :], _MASK_NEG, 0.0
    ).astype(np.float32)

    in_maps = []
    for c in range(_NC):
        b_idx = c // 2
        heads = (2 * (c % 2), 2 * (c % 2) + 1)
        xT = np.concatenate([x[b_idx].T, ones], axis=0)  # [65, 4096]
        wqk_cols = []
        wv_cols = []
        zeros16 = np.zeros((_D + 1, _Dh), np.float32)
        for h in heads:
            wqk_cols.append(aug(Wq, bq, h, _SCALE))
            wqk_cols.append(zeros16)
            wqk_cols.append(aug(Wk, bk, h))
            wv_cols.append(aug(Wv, bv, h))
        in_maps.append(
            {
                "xT": xT.astype(bf),
                "wqk": np.concatenate(wqk_cols, axis=1).astype(bf),
                "wv": np.concatenate(wv_cols, axis=1).astype(bf),
                "mask": mask,
            }
        )
    return in_maps


def _assemble(results):
    final = np.empty((_B, _S, _D), np.float32)
    for c in range(_NC):
        b_idx = c // 2
        for p in range(2):
            h = 2 * (c % 2) + p
            o = np.asarray(results[c]["out"], np.float32)  # [2, 17, S]
            final[b_idx, :, h * _Dh : (h + 1) * _Dh] = (o[p, :16] / o[p, 16:17]).T
    return final


def _run(in_maps, trace=False, trace_kwargs=None):
    from concourse.bass_utils import run_bass_kernel_spmd

    nc = _get_nc()
    return run_bass_kernel_spmd(
        nc, in_maps, list(range(_NC)), trace=trace, **(trace_kwargs or {})
    )


def kernel(x, Wq, bq, Wk, bk, Wv, bv):
    in_maps = _prepare_in_maps(x, Wq, bq, Wk, bk, Wv, bv)
    res = _run(in_maps)
    return _assemble(res.results)
